# revision 18
# baseline (speedup 1.0000x reference)
"""DiffJPEG Trainium2 Bass kernel.

Strategy (pure data-parallel over batch, 4 images per core on 8 cores):
  - load RGB planes in "rowpair" layout [128 rowpairs, 1024] (rows 2p,2p+1
    concatenated) so 2x2 avg-pooling never crosses partitions
  - RGB->Y via 2 fused scalar_tensor_tensor (Horner), chroma after pooling
  - 2D DCT as two matmul stages with a PE transpose between (H-transform
    contracts partitions directly; W-transform after 128x128 PE transposes)
  - quant scale (non-separable table part) via custom fused DVE op:
    out = r + (q*invT - r)^3, r = round-to-nearest-even via magic constant
  - dequant table part via one tensor_tensor multiply; all separable
    (0.25*alpha_u*alpha_v, color scales, /255, +-128 shifts) factors folded
    into matmul constants / per-partition bias vectors of PSUM->SBUF copies
  - IDCT mirrors DCT; chroma H-upsample folded into final matmul constants,
    W-upsample via strided copies; YCbCr->RGB via fused STT; clip via one
    2-op tensor_scalar
"""

import math
import os
import re

import numpy as np

import concourse.bacc as bacc
import concourse.bass as bass
import concourse.mybir as mybir
from concourse.mybir import ActivationFunctionType as Act, AluOpType as Op
from concourse.tile import TileContext

# --------------------------------------------------------------------------
# custom DVE op: out = diff_round(Src0 * Src1)
# --------------------------------------------------------------------------
import concourse.dve_ops as dve_ops
from concourse.dve_spec import C0, One, Spec, Src0, Src1, Zero, maxx, minn

MAGIC = float(np.float32(1.5 * 2**23))  # RNE rounding magic for |x| << 2^22


def _diffround_ref(in0, in1, s0, s1, imm2):
    m = (in0.astype(np.float32) * in1.astype(np.float32)).astype(np.float32)
    r = ((m + np.float32(s0)) - np.float32(s0)).astype(np.float32)
    e = (m - r).astype(np.float32)
    return (r + e * e * e).astype(np.float32)


_m = Src0 * Src1
_r = (_m + C0) - C0
_e = _m - _r
_DR_SPEC = Spec(body=_r + _e * _e * _e, reference=_diffround_ref)


def _clipaxpy_ref(in0, in1, s0, s1, imm2):
    v = (in0.astype(np.float32) + np.float32(s0) * in1.astype(np.float32))
    return np.clip(v, 0.0, 1.0).astype(np.float32)


_CA_SPEC = Spec(
    body=minn(maxx(Src0 + C0 * Src1, Zero), One), reference=_clipaxpy_ref
)


def _register_dve_op(name, spec):
    for op in dve_ops.OPS:
        if op.name == name:
            return op
    op = dve_ops.DveOp(name, spec, subdim=False, uops_sha={})
    dve_ops.OPS.append(op)
    dve_ops._SUB_OPCODE_FOR_NAME[name] = (
        dve_ops._CUSTOM_DVE_ROW_BASE + len(dve_ops.OPS) - 1
    )
    dve_ops.CUSTOM_DVE_SPECS[name] = spec
    for ver in ("v3", "v4"):
        try:
            op.compile(ver)
        except ValueError as e:
            m = re.search(r'="([0-9a-f]+)"', str(e))
            if m is None:
                raise
            op.uops_sha[ver] = m.group(1)
            op.compile(ver)
    return op


DIFFROUND = _register_dve_op("DIFF_ROUND_QANT", _DR_SPEC)
CLIPAXPY = _register_dve_op("CLIP_AXPY_01", _CA_SPEC)

# --------------------------------------------------------------------------
# constants
# --------------------------------------------------------------------------
P = 128
DT = mybir.dt.float32
NIMG = 4  # images per core
FACTOR = 0.4
# f32r mode: forward (stage1/stage2) risks diff_round boundary flips; the
# inverse path (iA/iB) is smooth so f32r there is ~1e-4-level noise only.
F32R_FWD = os.environ.get("KERNEL_F32R_FWD", "0") == "1"
F32R_INV = os.environ.get("KERNEL_F32R_INV", "1") == "1"
POOL_ON_GPSIMD = os.environ.get("KERNEL_POOL_GPSIMD", "1") == "1"
COLOR_ON_GPSIMD = os.environ.get("KERNEL_COLOR_GPSIMD", "0") == "1"
NO_CUSTOM = os.environ.get("KERNEL_NO_CUSTOM", "0") == "1"
PIPELINE = os.environ.get("KERNEL_PIPELINE", "1") == "1"
DEQ_ON_DVE = os.environ.get("KERNEL_DEQ_DVE", "1") == "1"
COLOR1_ON_GPSIMD = os.environ.get("KERNEL_COLOR1_GPSIMD", "0") == "1"

# constants packed into three tensors (always-fp32 / forward weights /
# inverse weights) -> one DMA + one sem each; weight groups take the dtype
# of their matmul path so the f32r producer-dtype rule is satisfied.
def _mk_layout(items):
    off_map, off = {}, 0
    for n, w in items:
        off_map[n] = (off, w)
        off += w
    return off_map, off


_CONST_OFF, _CTOT = _mk_layout(
    [
        ("ident", 128),
        ("q1y", 512),
        ("p2y", 512),
        ("q1c", 256),
        ("p2c", 256),
        ("bias_c1y", 1),
        ("bias_c4y", 1),
    ]
)
_CONSTF_OFF, _CFTOT = _mk_layout(
    [("w_s1y0", 128), ("w_s1y1", 128), ("w_s1c", 128), ("w_s2", 128)]
)
_CONSTI_OFF, _CITOT = _mk_layout(
    [("w_idct", 128), ("w_ibc0", 128), ("w_ibc1", 128), ("identr", 128)]
)

# color Horner ratios (float64 -> cast later)
_AY = 0.587 / 0.299
_BY = 0.114 / 0.587
_ACB = -0.331264 / 0.5
_BCB = -0.168736 / 0.5
_RCB = _BCB / _ACB
_ACR = -0.418688 / 0.5
_BCR = -0.081312 / 0.5
_RCR = _BCR / _ACR


def build_const_arrays(y_table, c_table):
    A = np.zeros((8, 8), np.float64)  # A[u,x] = cos((2x+1) u pi/16)
    for u in range(8):
        for x in range(8):
            A[u, x] = math.cos((2 * x + 1) * u * math.pi / 16)
    alpha = np.array([1.0 / math.sqrt(2)] + [1.0] * 7)
    Ah = (0.5 * alpha)[:, None] * A  # Ah[u,x] = 0.5*alpha_u*A[u,x]
    cY = 255.0 * 0.299
    cC = 0.5 * 255.0 / 4.0

    C = {}
    for s in (0, 1):
        # replicated in both partition halves so either 64-row slice works
        W = np.zeros((128, 128), np.float64)
        for rp in range(128):
            Ib, x = (rp % 64) // 4, 2 * (rp % 4) + s
            for u in range(8):
                W[rp, 8 * Ib + u] = Ah[u, x] * cY
        C[f"w_s1y{s}"] = W
    W = np.zeros((128, 128))
    for pr in range(128):
        Ib, x = pr // 8, pr % 8
        for u in range(8):
            W[pr, 8 * Ib + u] = Ah[u, x] * cC
    C["w_s1c"] = W
    W = np.zeros((128, 128))
    for wl in range(128):
        J, y = wl // 8, wl % 8
        for v in range(8):
            W[wl, 8 * J + v] = Ah[v, y]
    C["w_s2"] = W
    W = np.zeros((128, 128))
    for j in range(16):
        for v in range(8):
            for y in range(8):
                W[8 * j + v, 8 * j + y] = Ah[v, y]
    C["w_idct"] = W
    for par in (0, 1):
        W = np.zeros((128, 128))
        for p in range(128):
            xloc = 64 * par + p // 2
            Ib, x = xloc // 8, xloc % 8
            for u in range(8):
                W[8 * Ib + u, p] = Ah[u, x]
        C[f"w_ibc{par}"] = W
    C["ident"] = np.eye(128)
    C["identr"] = np.eye(128)

    def pats(T, ncols):
        T = np.asarray(T, np.float64)
        q1 = np.zeros((128, ncols))
        p2 = np.zeros((128, ncols))
        for p in range(128):
            v = p % 8
            for c in range(ncols):
                u = c % 8
                q1[p, c] = 1.0 / (T[u, v] * FACTOR)
                p2[p, c] = T[u, v] * FACTOR / 255.0
        return q1, p2

    C["q1y"], C["p2y"] = pats(y_table, 512)
    C["q1c"], C["p2c"] = pats(c_table, 256)

    b = np.zeros((128, 1))
    b[0::8, 0] = -1024.0 * 0.5 * alpha[0]
    C["bias_c1y"] = b
    b = np.zeros((128, 1))
    b[0::8, 0] = (128.0 / 255.0) / (0.5 * alpha[0])
    C["bias_c4y"] = b
    def pack(off_map, tot):
        p = np.zeros((128, tot), np.float32)
        for n, (off, w) in off_map.items():
            p[:, off : off + w] = np.asarray(C[n], np.float32)
        return p

    return pack(_CONST_OFF, _CTOT), pack(_CONSTF_OFF, _CFTOT), pack(_CONSTI_OFF, _CITOT)


# --------------------------------------------------------------------------
# program
# --------------------------------------------------------------------------
def build_program():
    FDT = mybir.dt.float32r if F32R_FWD else DT
    IDT = mybir.dt.float32r if F32R_INV else DT
    nc = bacc.Bacc("TRN2", target_bir_lowering=False)
    img = nc.dram_tensor("img", [NIMG, 3, 512, 512], DT, kind="ExternalInput")
    out = nc.dram_tensor("out", [NIMG, 3, 512, 512], DT, kind="ExternalOutput")
    cdram = nc.dram_tensor("consts", [128, _CTOT], DT, kind="ExternalInput")
    cfdram = nc.dram_tensor("constsf", [128, _CFTOT], FDT, kind="ExternalInput")
    cidram = nc.dram_tensor("constsi", [128, _CITOT], IDT, kind="ExternalInput")

    def mk(ap):
        return ap

    with TileContext(nc) as tc:
        with (
            tc.tile_pool(name="pc", bufs=1) as pc,
            tc.tile_pool(name="ps", bufs=8, space="PSUM") as ps,
            tc.tile_pool(name="pin", bufs=4) as pin,
            tc.tile_pool(name="py", bufs=2) as py,
            tc.tile_pool(name="php", bufs=2) as php,
            tc.tile_pool(name="pwp", bufs=4) as pwp,
            tc.tile_pool(name="pcc", bufs=3) as pcc,
            tc.tile_pool(name="pst1", bufs=6) as pst1,
            tc.tile_pool(name="pt2s", bufs=6) as pt2s,
            tc.tile_pool(name="pmid", bufs=4) as pmid,
            tc.tile_pool(name="pdeq", bufs=5) as pdeq,
            tc.tile_pool(name="pc3", bufs=5) as pc3,
            tc.tile_pool(name="pc4", bufs=6) as pc4,
            tc.tile_pool(name="pcup", bufs=4) as pcup,
            tc.tile_pool(name="prgb", bufs=6) as prgb,
        ):
            cwt = pc.tile([128, _CTOT], DT, tag="consts", name="t_consts")
            cwtf = pc.tile([128, _CFTOT], FDT, tag="constsf", name="t_constsf")
            cwti = pc.tile([128, _CITOT], IDT, tag="constsi", name="t_constsi")
            cw = {
                n: cwt[:, off : off + w] for n, (off, w) in _CONST_OFF.items()
            }
            cw.update(
                {n: cwtf[:, off : off + w] for n, (off, w) in _CONSTF_OFF.items()}
            )
            cw.update(
                {n: cwti[:, off : off + w] for n, (off, w) in _CONSTI_OFF.items()}
            )

            def emit_consts():
                # emitted after the first input loads so the 1.3MB of
                # tables does not delay image 0's color/pooling chain
                nc.sync.dma_start(out=cwt[:], in_=cdram[:])
                nc.sync.dma_start(out=cwtf[:], in_=cfdram[:])
                nc.sync.dma_start(out=cwti[:], in_=cidram[:])
                # warm DVE/ACT vector clocks past the const DMA so downstream
                # STT/custom-DVE instructions never carry the const-DMA wait
                # (the STT instruction struct encodes at most one sync wait)
                scr = pc.tile([1, 8], DT, tag="scr", name="scr0")
                nc.vector.tensor_copy(scr[0:1, 0:1], cwt[0:1, 0:1])
                nc.scalar.activation(scr[0:1, 1:2], cwt[0:1, 0:1], Act.Copy)

            eng_pool = nc.gpsimd if POOL_ON_GPSIMD else nc.vector
            eng_col = nc.gpsimd if COLOR_ON_GPSIMD else nc.vector
            eng_deq = nc.vector if DEQ_ON_DVE else nc.gpsimd
            eng_col1 = nc.gpsimd if COLOR1_ON_GPSIMD else eng_col

            import bass_rust as _br

            def dup2(ap):
                # read each column twice (W-upsample) via a step-0 dim
                return _br.AP(
                    tensor=ap.tensor,
                    offset=ap.offset,
                    ap=[list(ap.ap[0]), list(ap.ap[1]), [0, 2]],
                )

            S = [dict() for _ in range(NIMG)]  # per-image state

            def emit_load(b):
                # load rowpair tiles (one DMA per chunk, all 3 channels ->
                # single DMA sem for every consumer)
                planes = img[b].rearrange("c (rp s) w -> rp c (s w)", s=2)
                rpt = {}
                for k in range(2):
                    t = pin.tile([P, 3072], DT, tag="in", name=f"in{b}_{k}")
                    nc.sync.dma_start(
                        out=t[:].rearrange("p (c f) -> p c f", c=3),
                        in_=planes[128 * k : 128 * k + 128],
                    )
                    for c in range(3):
                        rpt[c, k] = t[:, 1024 * c : 1024 * c + 1024]
                S[b]["rpt"] = rpt

            def emit_front(b):
                rpt = S[b]["rpt"]
                # ---------------- Y color (Horner STT) ----------------
                yt = {}
                for k in range(2):
                    t1 = py.tile([P, 1024], DT, tag="yt1", name=f"yt1_{b}{k}")
                    eng_col1.scalar_tensor_tensor(
                        t1[:], rpt[2, k][:], _BY, rpt[1, k][:], Op.mult, Op.add
                    )
                    t2 = py.tile([P, 1024], FDT, tag="yt2", name=f"yt2_{b}{k}")
                    eng_col.scalar_tensor_tensor(
                        t2[:], t1[:], _AY, rpt[0, k][:], Op.mult, Op.add
                    )
                    yt[k] = t2

                # ---------------- 2x2 pooling ----------------
                pooled = {}
                for k in range(2):
                    hp = php.tile([P, 1536], DT, tag="hp", name=f"hp{b}_{k}")
                    src3 = (
                        rpt[0, k]
                        .tensor[:, :]
                        .rearrange("p (c f) -> p c f", c=3)
                    )
                    eng_pool.tensor_tensor(
                        hp[:].rearrange("p (c f) -> p c f", c=3),
                        src3[:, :, 0:512],
                        src3[:, :, 512:1024],
                        Op.add,
                    )
                    wp = pwp.tile([P, 768], DT, tag="wp", name=f"wp{b}_{k}")
                    hp3 = hp[:].rearrange("p (c f) -> p c f", c=3)
                    eng_pool.tensor_tensor(
                        wp[:].rearrange("p (c f) -> p c f", c=3),
                        hp3[:, :, 0:512:2],
                        hp3[:, :, 1:512:2],
                        Op.add,
                    )
                    for c in range(3):
                        pooled[c, k] = wp[:, 256 * c : 256 * c + 256]

                # ---------------- chroma color ----------------
                cbcr = {}
                for k in range(2):
                    t1 = pcc.tile([P, 256], DT, tag="cct", name=f"cbt{b}{k}")
                    eng_col1.scalar_tensor_tensor(
                        t1[:], pooled[0, k][:], _RCB, pooled[1, k][:], Op.mult, Op.add
                    )
                    cb = pcc.tile([P, 256], FDT, tag="cb", name=f"cb{b}{k}")
                    eng_col.scalar_tensor_tensor(
                        cb[:], t1[:], _ACB, pooled[2, k][:], Op.mult, Op.add
                    )
                    t2c = pcc.tile([P, 256], DT, tag="cct", name=f"crt{b}{k}")
                    eng_col1.scalar_tensor_tensor(
                        t2c[:], pooled[2, k][:], _RCR, pooled[1, k][:], Op.mult, Op.add
                    )
                    cr = pcc.tile([P, 256], FDT, tag="cr", name=f"cr{b}{k}")
                    eng_col.scalar_tensor_tensor(
                        cr[:], t2c[:], _ACR, pooled[0, k][:], Op.mult, Op.add
                    )
                    cbcr["cb", k] = cb
                    cbcr["cr", k] = cr

                # ---------------- stage 1 (H transform) ----------------
                st1 = {}
                for m in range(4):
                    pt = ps.tile([P, 512], DT, tag="ps", name=f"p_s1y{b}{m}")
                    k, off = m // 2, 64 * (m % 2)
                    nc.tensor.matmul(
                        pt[:],
                        mk(cw["w_s1y0"][off : off + 64, :]),
                        mk(yt[k][off : off + 64, 0:512]),
                        start=True,
                        stop=False,
                    )
                    nc.tensor.matmul(
                        pt[:],
                        mk(cw["w_s1y1"][off : off + 64, :]),
                        mk(yt[k][off : off + 64, 512:1024]),
                        start=False,
                        stop=True,
                    )
                    s = pst1.tile([P, 512], DT, tag="st1", name=f"st1y{b}{m}")
                    nc.scalar.activation(
                        s[:], pt[:], Act.Identity, bias=cw["bias_c1y"][:, 0:1]
                    )
                    st1["y", m] = s
                for ch in ("cb", "cr"):
                    for k in range(2):
                        pt = ps.tile([P, 256], DT, tag="ps", name=f"p_s1{ch}{b}{k}")
                        nc.tensor.matmul(
                            pt[:],
                            mk(cw["w_s1c"][:]),
                            mk(cbcr[ch, k][:]),
                            start=True,
                            stop=True,
                        )
                        s = pst1.tile([P, 256], DT, tag="st1", name=f"st1{ch}{b}{k}")
                        nc.scalar.activation(s[:], pt[:], Act.Copy)
                        st1[ch, k] = s

                # ---------------- T1 transpose + c2 ----------------
                t2s = {}
                for j in range(4):
                    pt = ps.tile([P, 512], DT, tag="ps", name=f"p_t1y{b}{j}")
                    for m in range(4):
                        nc.tensor.transpose(
                            pt[:, 128 * m : 128 * m + 128],
                            st1["y", m][:, 128 * j : 128 * j + 128],
                            cw["ident"][:],
                        )
                    s = pt2s.tile([P, 512], FDT, tag="t2s", name=f"t2sy{b}{j}")
                    nc.scalar.activation(s[:], pt[:], Act.Copy)
                    t2s["y", j] = s
                for ch in ("cb", "cr"):
                    for jc in range(2):
                        pt = ps.tile([P, 256], DT, tag="ps", name=f"p_t1{ch}{b}{jc}")
                        for mp in range(2):
                            nc.tensor.transpose(
                                pt[:, 128 * mp : 128 * mp + 128],
                                st1[ch, mp][:, 128 * jc : 128 * jc + 128],
                                cw["ident"][:],
                            )
                        s = pt2s.tile([P, 256], FDT, tag="t2s", name=f"t2s{ch}{b}{jc}")
                        nc.scalar.activation(s[:], pt[:], Act.Copy)
                        t2s[ch, jc] = s

                # ---------------- stage 2 + quant/diff_round/dequant ----------------
                deq = {}
                for key, q1, p2, w in (
                    ("y", "q1y", "p2y", 512),
                    ("cb", "q1c", "p2c", 256),
                    ("cr", "q1c", "p2c", 256),
                ):
                    nj = 4 if key == "y" else 2
                    for j in range(nj):
                        pt = ps.tile([P, w], DT, tag="ps", name=f"p_s2{key}{b}{j}")
                        nc.tensor.matmul(
                            pt[:],
                            mk(cw["w_s2"][:]),
                            mk(t2s[key, j][:]),
                            start=True,
                            stop=True,
                        )
                        ymid = pmid.tile([P, w], DT, tag="ymid", name=f"md{key}{b}{j}")
                        if NO_CUSTOM:
                            tm = pmid.tile([P, w], DT, tag="tm", bufs=2, name=f"tm{key}{b}{j}")
                            nc.vector.tensor_tensor(
                                tm[:], pt[:], cw[q1][:, 0:w], Op.mult
                            )
                            tr = pmid.tile([P, w], DT, tag="tr", bufs=2, name=f"tr{key}{b}{j}")
                            nc.vector.tensor_scalar(
                                tr[:], tm[:], MAGIC, -MAGIC, Op.add, Op.add
                            )
                            te = pmid.tile([P, w], DT, tag="te", bufs=2, name=f"te{key}{b}{j}")
                            nc.vector.tensor_tensor(te[:], tm[:], tr[:], Op.subtract)
                            t3 = pmid.tile([P, w], DT, tag="t3", bufs=2, name=f"t3{key}{b}{j}")
                            nc.vector.tensor_tensor(t3[:], te[:], te[:], Op.mult)
                            nc.vector.tensor_tensor(t3[:], t3[:], te[:], Op.mult)
                            nc.vector.tensor_tensor(ymid[:], tr[:], t3[:], Op.add)
                        else:
                            nc.vector._custom_dve(
                                DIFFROUND,
                                out=ymid[:],
                                in0=pt[:],
                                in1=cw[q1][:, 0:w],
                                s0=MAGIC,
                            )
                        d = pdeq.tile(
                            [P, w], IDT, tag=f"deq{key}",
                            bufs=6 if key == "y" else 3, name=f"dq{key}{b}{j}"
                        )
                        # chroma dequant rides the (mostly idle) gpsimd; the
                        # y dequant stays on DVE so gpsimd never blocks the
                        # next image's pooling behind this image's forward
                        eng_d = eng_deq if key == "y" else nc.gpsimd
                        eng_d.tensor_tensor(d[:], ymid[:], cw[p2][:, 0:w], Op.mult)
                        deq[key, j] = d
                S[b]["deq"] = deq

            def emit_back(b):
                deq = S[b]["deq"]
                # ---------------- iA (inverse W) + c3 ----------------
                c3 = {}
                for key, w in (("y", 512), ("cb", 256), ("cr", 256)):
                    nj = 4 if key == "y" else 2
                    for j in range(nj):
                        pt = ps.tile([P, w], DT, tag="ps", name=f"p_ia{key}{b}{j}")
                        nc.tensor.matmul(
                            pt[:],
                            mk(cw["w_idct"][:]),
                            mk(deq[key, j][:]),
                            start=True,
                            stop=True,
                        )
                        s = pc3.tile([P, w], IDT, tag="c3", name=f"c3{key}{b}{j}")
                        nc.scalar.activation(s[:], pt[:], Act.Copy)
                        c3[key, j] = s

                # ---------------- T2 transpose + c4 (f32r: inverse path
                # tolerates the 12-bit moving-data truncation) ----------------
                TDT = IDT
                tid = "identr" if IDT != DT else "ident"
                c4 = {}
                for m in range(4):
                    pt = ps.tile([P, 512], TDT, tag="ps", name=f"p_t2y{b}{m}")
                    for j in range(4):
                        nc.tensor.transpose(
                            pt[:, 128 * j : 128 * j + 128],
                            c3["y", j][:, 128 * m : 128 * m + 128],
                            cw[tid][:],
                        )
                    s = pc4.tile([P, 512], IDT, tag="c4", name=f"c4y{b}{m}")
                    nc.scalar.activation(
                        s[:], pt[:], Act.Identity, bias=cw["bias_c4y"][:, 0:1]
                    )
                    c4["y", m] = s
                for ch in ("cb", "cr"):
                    for mp in range(2):
                        pt = ps.tile([P, 256], TDT, tag="ps", name=f"p_t2{ch}{b}{mp}")
                        for jc in range(2):
                            nc.tensor.transpose(
                                pt[:, 128 * jc : 128 * jc + 128],
                                c3[ch, jc][:, 128 * mp : 128 * mp + 128],
                                cw[tid][:],
                            )
                        s = pc4.tile([P, 256], IDT, tag="c4", name=f"c4{ch}{b}{mp}")
                        nc.scalar.activation(s[:], pt[:], Act.Copy)
                        c4[ch, mp] = s

                # ---------------- iB + upsample + recombine + clip + store --------
                _GR = 0.344136 / 0.714136
                for mo in range(4):
                    ypt = ps.tile([P, 512], DT, tag="ps", name=f"p_iby{b}{mo}")
                    nc.tensor.matmul(
                        ypt[:],
                        mk(cw["w_idct"][:]),
                        mk(c4["y", mo][:]),
                        start=True,
                        stop=True,
                    )
                    cq = {}
                    for ch in ("cb", "cr"):
                        cpt = ps.tile([P, 256], DT, tag="ps", name=f"p_ib{ch}{b}{mo}")
                        nc.tensor.matmul(
                            cpt[:],
                            mk(cw[f"w_ibc{mo % 2}"][:]),
                            mk(c4[ch, mo // 2][:]),
                            start=True,
                            stop=True,
                        )
                        q = pcup.tile([P, 256], DT, tag="cup", name=f"cu{ch}{b}{mo}")
                        nc.scalar.activation(q[:], cpt[:], Act.Copy)
                        cq[ch] = q

                    rows = slice(128 * mo, 128 * mo + 128)
                    # absorber: pull the PE-sem wait onto a 1x1 copy so the
                    # following STT carries at most one sync wait
                    ab = prgb.tile([1, 1], DT, tag="ab", name=f"ab{b}{mo}")
                    nc.vector.tensor_copy(ab[0:1, 0:1], ypt[0:1, 0:1])
                    # one [P,1536] tile holds R|G|B; recombine + clip fused in
                    # a single custom DVE pass per channel
                    rgb = prgb.tile([P, 1536], DT, tag="rgb", bufs=2, name=f"rgb{b}{mo}")
                    nc.vector._custom_dve(
                        CLIPAXPY, out=rgb[:, 0:512], in0=ypt[:],
                        in1=dup2(cq["cr"][:]), s0=1.402,
                    )
                    # G = Y - 0.714136*(cr + (0.344136/0.714136)*cb), combined
                    # at quarter resolution first
                    gq = pcup.tile([P, 256], DT, tag="gq", name=f"gq{b}{mo}")
                    nc.vector.scalar_tensor_tensor(
                        gq[:], cq["cb"][:], _GR, cq["cr"][:], Op.mult, Op.add
                    )
                    nc.vector._custom_dve(
                        CLIPAXPY, out=rgb[:, 512:1024], in0=ypt[:],
                        in1=dup2(gq[:]), s0=-0.714136,
                    )
                    nc.vector._custom_dve(
                        CLIPAXPY, out=rgb[:, 1024:1536], in0=ypt[:],
                        in1=dup2(cq["cb"][:]), s0=1.772,
                    )
                    nc.sync.dma_start(
                        out=out[b][:, rows, :].rearrange("c h w -> h c w"),
                        in_=rgb[:].rearrange("p (c f) -> p c f", c=3),
                    )

            # ---------------- emission schedule ----------------
            # depth-2 software pipeline: while image b's inverse half runs,
            # image b+1's forward half keeps every engine fed, and image
            # b+2's input DMA is already in flight.
            if PIPELINE:
                emit_consts()
                emit_load(0)
                if NIMG > 1:
                    emit_load(1)
                emit_front(0)
                for b in range(1, NIMG):
                    if b + 1 < NIMG:
                        emit_load(b + 1)
                    emit_front(b)
                    emit_back(b - 1)
                emit_back(NIMG - 1)
            else:
                emit_consts()
                for b in range(NIMG):
                    emit_load(b)
                    emit_front(b)
                    emit_back(b)

    nc.compile()
    return nc


# --------------------------------------------------------------------------
# entry point
# --------------------------------------------------------------------------
_last_results = None


def kernel(image, y_table, c_table):
    global _last_results
    from concourse import bass_utils

    image = np.ascontiguousarray(np.asarray(image), np.float32)
    packed, packedf, packedi = build_const_arrays(
        np.asarray(y_table), np.asarray(c_table)
    )

    nc = build_program()
    n_cores = 8
    per = image.shape[0] // n_cores
    in_maps = [
        {
            "img": np.ascontiguousarray(image[i * per : (i + 1) * per]),
            "consts": packed,
            "constsf": packedf,
            "constsi": packedi,
        }
        for i in range(n_cores)
    ]

    res = None
    last_exc = None
    for attempt in range(3):
        try:
            res = bass_utils.run_bass_kernel_spmd(
                nc,
                in_maps,
                core_ids=list(range(n_cores)),
                trace=os.environ.get("KERNEL_TRACE", "0") == "1",
            )
            break
        except Exception as e:  # transient NRT/device hiccups: retry
            last_exc = e
    if res is None:
        raise last_exc
    _last_results = res
    outs = [r["out"] for r in res.results]
    return np.concatenate(outs, axis=0).astype(np.float32)


if __name__ == "__main__":
    rng = np.random.default_rng(0)
    img = rng.random((32, 3, 512, 512), np.float32)
    yt = np.ones((8, 8), np.float32)
    ct = np.ones((8, 8), np.float32)
    out = kernel(img, yt, ct)
    print("out", out.shape, out.dtype, float(out.min()), float(out.max()))

